# revision 1
# baseline (speedup 1.0000x reference)
"""OHEM cross-entropy loss (CriterionOhem) on 8 Trainium2 NeuronCores.

Reference semantics (N = 4*512*1024 pixels, C = 19 classes):
  p_i     = softmax(pred)[i, t_i]                (true-class prob per pixel)
  kth     = sort(p)[MIN_KEPT-1]
  thr     = max(kth, 0.7)
  keep_i  = p_i <= thr
  loss    = sum(keep_i * nll_i) / max(count(keep), 1)

Key reduction: if count(p <= 0.7) >= MIN_KEPT then kth <= 0.7 and thr == 0.7
exactly, so no top-k is needed — only a masked sum + count, which the host
verifies from the device partials (falling back to a numpy reference in the
degenerate case, which cannot occur for randn logits).

Device layout per core (262,144 pixels), all fp16 on the wire:
  22 tiles [19C x B blocks, 2X] with partition p = j + B*c (c-major, B=6
  main / B=2 tail).  Each tile is ONE combined DMA: columns [0,X) hold the
  logits, columns [X,2X) hold the target pre-shifted by the partition's
  class ((t - c), so the on-device compare is against 0.0).  ACT computes
  E=exp(x), DVE computes the masked-logit gm = (t-c==0)*x in a single
  scalar_tensor_tensor, and PE contracts the 19 classes with constant
  one-hot block maps:
      S[pixel]  = sum_c e^x   (softmax denominator)
      Xt[pixel] = x_t         (target-class logit)
  accumulated in PSUM groups of 16 tiles (96 rows) + 6 tiles (32 rows).
  Final pass: nll = ln S - Xt, keep = (nll >= -ln 0.7), per-partition
  masked sum + count -> [128, 2] partial per core; host combines.
"""

import numpy as np
from contextlib import ExitStack

import concourse.bass as bass
import concourse.tile as tile
from concourse import bacc
from concourse import mybir
from concourse.bass_utils import run_bass_kernel_spmd

F32 = mybir.dt.float32
F16 = mybir.dt.float16
I8 = mybir.dt.int8
U8 = mybir.dt.uint8
AF = mybir.ActivationFunctionType
OP = mybir.AluOpType

C = 19
THRESH = 0.7
MIN_KEPT = 100000
NLL_MIN = float(-np.log(np.float32(THRESH)))  # keep <=> nll >= -ln(0.7)

# Full-size geometry: 4x19x512x1024 pred over 8 cores.
BATCH, HH, WW = 4, 512, 1024
NCORES = 8


class Geo:
    def __init__(self, X, nmain, tailb):
        self.X = X                      # free elems per tile (pixels)
        self.NMAIN = nmain              # number of 6-block main tiles
        self.TAILB = tailb              # blocks in the tail tile
        self.PSTAGE = 6 * nmain + tailb # stage partitions (== pixel rows)
        self.NPIX = self.PSTAGE * X     # pixels per core


GEO_FULL = Geo(2048, 21, 2)             # 128 x 2048 = 262,144 pixels/core


def _grouping(g):
    """PE accumulation groups: A = first GA tiles -> MA psum rows (must be a
    multiple of 32 for engine start-partition rules => GA = 16), B = rest +
    tail -> MB rows at offset MA."""
    GA = min(16, g.NMAIN)
    MA = 6 * GA
    MB = 6 * (g.NMAIN - GA) + g.TAILB
    assert MA <= 128 and 0 < MB <= 128
    return GA, MA, MB


def make_consts(g):
    """Stacked one-hot block maps (c-major partitions p = j + B*c).  Map for
    tile i within its group is the [P, M] slice at columns [i*M, (i+1)*M),
    with ones at (j+B*c, 6*i+j)."""
    GA, MA, MB = _grouping(g)
    NB = g.NMAIN - GA

    maps_a = np.zeros((6 * C, GA, MA), np.float16)
    for i in range(GA):
        for p in range(6 * C):
            maps_a[p, i, 6 * i + p % 6] = 1.0

    maps_b = np.zeros((6 * C, max(NB, 1), MB), np.float16)
    for i in range(NB):
        for p in range(6 * C):
            maps_b[p, i, 6 * i + p % 6] = 1.0

    map_tail = np.zeros((g.TAILB * C, MB), np.float16)
    for p in range(g.TAILB * C):
        map_tail[p, 6 * NB + p % g.TAILB] = 1.0

    packed = np.zeros((6 * C, GA * MA + max(NB, 1) * MB + MB), np.float16)
    packed[:, :GA * MA] = maps_a.reshape(6 * C, GA * MA)
    packed[:, GA * MA:GA * MA + max(NB, 1) * MB] = maps_b.reshape(
        6 * C, max(NB, 1) * MB)
    packed[:g.TAILB * C, GA * MA + max(NB, 1) * MB:] = map_tail
    return {"maps": packed}


def emit(ctx, tc, g, pk, maps, acc):
    nc = tc.nc
    X = g.X
    NSLC = X // 512
    GA, MA, MB = _grouping(g)
    NB = g.NMAIN - GA

    cmb = ctx.enter_context(tc.tile_pool(name="cmb", bufs=3))
    ep = ctx.enter_context(tc.tile_pool(name="ep", bufs=3))
    gmp = ctx.enter_context(tc.tile_pool(name="gmp", bufs=3))
    cst = ctx.enter_context(tc.tile_pool(name="cst", bufs=1))
    pss = ctx.enter_context(tc.tile_pool(name="pss", bufs=1, space="PSUM"))
    fin = ctx.enter_context(tc.tile_pool(name="fin", bufs=1))

    # One packed constant tensor: [maps_a | maps_b | map_tail] columns.
    CA = GA * MA
    CB = max(NB, 1) * MB
    maps_t = cst.tile([6 * C, CA + CB + MB], F16)
    nc.sync.dma_start(maps_t[:], maps)

    # Final-pass tiles.  The finals for each PSUM group run straight out of
    # PSUM (no evacuation copies): ln(S) on ACT, nll/count/num on DVE with
    # fused per-partition accumulation.  num = sum relu(nll - t) + t*cnt.
    lnS = fin.tile([g.PSTAGE, X], F32)
    nll = fin.tile([g.PSTAGE, X], F32)
    k = fin.tile([g.PSTAGE, X], F32)
    rl = fin.tile([g.PSTAGE, X], F32)
    acc2 = fin.tile([g.PSTAGE, 4], F32)
    nc.vector.memset(acc2[:], 0.0)

    def finals(nr, S_ps, T_ps, col):
        r = slice(0, nr)
        nc.scalar.activation(lnS[r, :], S_ps[r, :], AF.Ln)
        nc.vector.tensor_sub(nll[r, :], lnS[r, :], T_ps[r, :])
        nc.vector.tensor_scalar(
            out=k[r, :], in0=nll[r, :], scalar1=NLL_MIN, scalar2=None,
            op0=OP.is_ge)
        nc.vector.tensor_reduce(acc2[r, col + 1:col + 2], k[r, :],
                                axis=mybir.AxisListType.X, op=OP.add)
        nc.vector.tensor_scalar(
            out=rl[r, :], in0=nll[r, :], scalar1=-NLL_MIN, scalar2=0.0,
            op0=OP.add, op1=OP.max)
        nc.vector.tensor_reduce(acc2[r, col:col + 1], rl[r, :],
                                axis=mybir.AxisListType.X, op=OP.add)

    # Both PSUM accumulator pairs are [128, X]; group A uses rows 0:MA,
    # group B rows MA:MA+MB (so engine ops see matching partition bases).
    S_ps = T_ps = None
    for i in range(g.NMAIN + 1):
        is_tail = i == g.NMAIN
        Bb = g.TAILB if is_tail else 6
        P = C * Bb
        prow = 6 * i

        if i == 0:
            S_ps = pss.tile([128, X], F32, tag="s")
            T_ps = pss.tile([128, X], F32, tag="g")
        elif i == GA:
            # Group-A finals read from PSUM; banks free afterwards and are
            # reallocated for group B.
            finals(MA, S_ps, T_ps, 0)
            S_ps = pss.tile([128, X], F32, tag="s")
            T_ps = pss.tile([128, X], F32, tag="g")

        # One packed DMA per tile: [x as fp16 bytes | t-c as int8].
        ptile = cmb.tile([P, 3 * X], U8, tag="c")
        nc.sync.dma_start(ptile[:], pk[:, prow:prow + Bb, :])
        xv = ptile[:, 0:2 * X].bitcast(F16)
        tv = ptile[:, 2 * X:3 * X].bitcast(I8)

        et = ep.tile([P, X], F16, tag="e")
        nc.scalar.activation(et[:], xv, AF.Exp)

        gm = gmp.tile([P, X], F16, tag="g")
        nc.vector.scalar_tensor_tensor(
            out=gm[:],
            in0=tv,
            scalar=0.0,
            in1=xv,
            op0=OP.is_equal,
            op1=OP.mult,
        )

        if is_tail:
            lhs = maps_t[0:g.TAILB * C, CA + CB:CA + CB + MB]
        elif i < GA:
            lhs = maps_t[:, i * MA:(i + 1) * MA]
        else:
            lhs = maps_t[:, CA + (i - GA) * MB:CA + (i - GA + 1) * MB]
        first = (i == 0) or (i == GA)
        last = (i == GA - 1) or is_tail
        rows = slice(0, MA) if i < GA else slice(0, MB)
        for m in range(NSLC):
            sl = slice(m * 512, (m + 1) * 512)
            nc.tensor.matmul(S_ps[rows, sl], lhs, et[:, sl],
                             start=first, stop=last)
            nc.tensor.matmul(T_ps[rows, sl], lhs, gm[:, sl],
                             start=first, stop=last)

    # Group-B finals.
    finals(MB, S_ps, T_ps, 2)

    nc.sync.dma_start(acc[:, :], acc2[:])


def build_nc(g):
    nc = bacc.Bacc(
        "TRN2",
        target_bir_lowering=False,
        debug=False,
        enable_asserts=True,
        num_devices=NCORES,
    )
    GA, MA, MB = _grouping(g)
    NB = g.NMAIN - GA
    pk = nc.dram_tensor("pk", [C, g.PSTAGE, 3 * g.X], U8, kind="ExternalInput")
    maps = nc.dram_tensor(
        "maps", [6 * C, GA * MA + max(NB, 1) * MB + MB], F16,
        kind="ExternalInput")
    acc = nc.dram_tensor("acc", [g.PSTAGE, 4], F32, kind="ExternalOutput")
    with tile.TileContext(nc) as tc, ExitStack() as ctx:
        emit(ctx, tc, g, pk.ap(), maps.ap(), acc.ap())
    nc.compile()
    return nc


_NC_CACHE = {}


def _get_nc(g):
    key = (g.X, g.NMAIN, g.TAILB)
    if key not in _NC_CACHE:
        _NC_CACHE[key] = build_nc(g)
    return _NC_CACHE[key]


def make_inputs(pred_slice, target_slice, g):
    """Per-core packed device input [C, PSTAGE, 3X] uint8: per row, the X
    fp16 logits as raw bytes followed by X int8 class-shifted targets
    (t - c)."""
    pkb = np.empty((C, g.PSTAGE, 3 * g.X), np.uint8)
    xc = pkb[:, :, :2 * g.X].view(np.float16)
    xc[:] = pred_slice.reshape(C, g.PSTAGE, g.X)
    tr = pkb[:, :, 2 * g.X:].view(np.int8)
    t1 = target_slice.astype(np.int8).reshape(g.PSTAGE, g.X)
    for c in range(C):
        np.subtract(t1, np.int8(c), out=tr[c])
    return pkb


def _shard_inputs(pred, target, g):
    """Slice the full inputs into per-core in_maps (8 cores)."""
    consts = make_consts(g)
    in_maps = []
    rows_per_core = HH // 2  # 256
    for core in range(NCORES):
        b, half = core // 2, core % 2
        h0 = half * rows_per_core
        pkb = make_inputs(pred[b, :, h0:h0 + rows_per_core, :],
                          target[b, h0:h0 + rows_per_core, :], g)
        m = {"pk": pkb}
        m.update(consts)
        in_maps.append(m)
    return in_maps


def _reference_numpy(pred, target):
    """Full numpy fallback with reference semantics (degenerate cases only)."""
    b, c, h, w = pred.shape
    n = b * h * w
    t = target.reshape(-1).astype(np.int64)
    valid = t != 255
    t0 = np.where(valid, t, 0)
    logits = np.transpose(pred, (0, 2, 3, 1)).reshape(n, c).astype(np.float32)
    m = logits.max(axis=1, keepdims=True)
    ex = np.exp(logits - m)
    s = ex.sum(axis=1)
    pt = ex[np.arange(n), t0] / s
    mask_prob = np.where(valid, pt, 1.0).astype(np.float32)
    kth = np.sort(mask_prob)[min(n, MIN_KEPT) - 1]
    thr = max(float(kth), THRESH)
    kept = mask_prob <= thr
    fv = valid & kept
    nll = (np.log(s) + m[:, 0] - logits[np.arange(n), t0]).astype(np.float32)
    num = float(np.where(fv, nll, 0.0).sum(dtype=np.float64))
    cnt = float(fv.sum())
    return np.float32(num / max(cnt, 1.0))


def _run_device(in_maps, g, trace=False):
    nc = _get_nc(g)
    return run_bass_kernel_spmd(nc, in_maps, list(range(NCORES)), trace=trace)


def kernel(pred, target):
    pred = np.asarray(pred)
    target = np.asarray(target)
    assert pred.shape == (BATCH, C, HH, WW), pred.shape
    assert target.shape == (BATCH, HH, WW), target.shape

    if target.min() < 0 or target.max() >= C:
        # ignore_index / out-of-range labels: not producible by the input
        # spec (randint 0..18); handle via the host reference for safety.
        return _reference_numpy(pred, target)

    g = GEO_FULL
    in_maps = _shard_inputs(pred, target, g)
    res = _run_device(in_maps, g).results

    num = 0.0
    cnt = 0.0
    for core in range(NCORES):
        a = res[core]["acc"].astype(np.float64)
        num += a[:, 0].sum() + a[:, 2].sum()
        cnt += a[:, 1].sum() + a[:, 3].sum()
    num += NLL_MIN * cnt

    if cnt < MIN_KEPT:
        # kth-smallest prob exceeds 0.7: threshold is data-dependent.
        return _reference_numpy(pred, target)

    return np.float32(num / max(cnt, 1.0))



# revision 2
# speedup vs baseline: 1.3984x; 1.3984x over previous
"""OHEM cross-entropy loss (CriterionOhem) on 8 Trainium2 NeuronCores.

Reference semantics (N = 4*512*1024 pixels, C = 19 classes):
  p_i     = softmax(pred)[i, t_i]                (true-class prob per pixel)
  kth     = sort(p)[MIN_KEPT-1]
  thr     = max(kth, 0.7)
  keep_i  = p_i <= thr
  loss    = sum(keep_i * nll_i) / max(count(keep), 1)

Key reduction: if count(p <= 0.7) >= MIN_KEPT then kth <= 0.7 and thr == 0.7
exactly, so no top-k is needed — only a masked sum + count, which the host
verifies from the device partials (falling back to a numpy reference in the
degenerate case, which cannot occur for randn logits).

Host-side trick: per pixel, the target-class logit is swapped into class
slot 0 (a pure permutation — the softmax denominator is permutation
invariant and x_t becomes the slot-0 plane).  This removes every trace of
the target from the device program: no replicated (t - c) bytes on the
wire, no masked-logit pass on DVE, no second matmul pass on PE.

Device layout per core (262,144 pixels = 128 blocks x 2048):
  xk  [128*19, 2048] fp16   class-rows, row g = 19*block + class
  x0  [128, 2048]    fp16   slot-0 (= target-class) logits per block
  19 tiles of 128 consecutive class-rows stream through:
    ACT: et = exp(x)   (tiles merged in pairs to amortize fixed overhead)
    PE : one-hot block maps contract the 19 classes of each block into
         S[block, pixel], accumulated across all 19 tiles into a single
         [128, 2048] fp32 PSUM region (4 banks, start@k=0 / stop@k=18).
  Finals: lnS = ln(S) on ACT; nll = lnS - x0, rl = relu(nll - t),
  keep = (nll >= t), per-partition sums -> [128, 2] partial; host combines
  num = sum(rl) + t*cnt, cnt = sum(keep).
"""

import numpy as np
from contextlib import ExitStack

import concourse.bass as bass
import concourse.tile as tile
from concourse import bacc
from concourse import mybir
from concourse.bass_utils import run_bass_kernel_spmd

F32 = mybir.dt.float32
F16 = mybir.dt.float16
AF = mybir.ActivationFunctionType
OP = mybir.AluOpType

C = 19
THRESH = 0.7
MIN_KEPT = 100000
NLL_MIN = float(-np.log(np.float32(THRESH)))  # keep <=> nll >= -ln(0.7)

# Full-size geometry: 4x19x512x1024 pred over 8 cores.
BATCH, HH, WW = 4, 512, 1024
NCORES = 8


class Geo:
    def __init__(self, X, nblk):
        self.X = X                      # pixels per block (free axis)
        self.NBLK = nblk                # blocks per core (psum partitions)
        self.NROWS = C * nblk           # class-rows per core
        self.NT = self.NROWS // 128     # 128-row tiles
        assert self.NROWS % 128 == 0
        self.NPIX = nblk * X            # pixels per core


GEO_FULL = Geo(2048, 128)               # 128 x 2048 = 262,144 pixels/core


def make_consts(g):
    """Per-tile one-hot block maps, packed side by side: map k column block
    [k*128,(k+1)*128) has a one at (p, (128k+p)//19)."""
    maps = np.zeros((128, g.NT * 128), np.float16)
    for k in range(g.NT):
        for p in range(128):
            maps[p, k * 128 + (128 * k + p) // C] = 1.0
    return {"maps": maps}


def emit(ctx, tc, g, xk, x0, maps, acc):
    nc = tc.nc
    X = g.X
    NSLC = X // 512

    # Tile pairs share one SBUF tile so each ACT exp covers 2*X columns.
    chunks = [(2 * i, 2 * i + 1) for i in range(g.NT // 2)]
    if g.NT % 2:
        chunks.append((g.NT - 1,))

    xvp = ctx.enter_context(tc.tile_pool(name="xvp", bufs=3))
    etp = ctx.enter_context(tc.tile_pool(name="etp", bufs=3))
    cst = ctx.enter_context(tc.tile_pool(name="cst", bufs=1))
    pss = ctx.enter_context(tc.tile_pool(name="pss", bufs=1, space="PSUM"))
    fin = ctx.enter_context(tc.tile_pool(name="fin", bufs=1))

    maps_t = cst.tile([128, g.NT * 128], F16)
    nc.sync.dma_start(maps_t[:], maps)
    x0t = cst.tile([128, X], F16)
    nc.sync.dma_start(x0t[:], x0)

    S_ps = pss.tile([128, X], F32)

    for ch in chunks:
        W = len(ch) * X
        xv = xvp.tile([128, W], F16, tag="x")
        for j, k in enumerate(ch):
            nc.sync.dma_start(xv[:, j * X:(j + 1) * X],
                              xk[128 * k:128 * (k + 1), :])
        et = etp.tile([128, W], F16, tag="e")
        nc.scalar.activation(et[:], xv[:], AF.Exp)
        for j, k in enumerate(ch):
            lhs = maps_t[:, k * 128:(k + 1) * 128]
            for m in range(NSLC):
                sl = slice(m * 512, (m + 1) * 512)
                nc.tensor.matmul(S_ps[:, sl], lhs,
                                 et[:, j * X + m * 512:j * X + (m + 1) * 512],
                                 start=(k == 0), stop=(k == g.NT - 1))

    # Finals: nll = ln S - x0; rl = relu(nll - t); keep = (nll >= t).
    lnS = fin.tile([128, X], F16)
    nc.scalar.activation(lnS[:], S_ps[:], AF.Ln)
    nll = fin.tile([128, X], F16)
    nc.vector.tensor_sub(nll[:], lnS[:], x0t[:])
    rl = fin.tile([128, X], F16)
    nc.vector.tensor_scalar(out=rl[:], in0=nll[:], scalar1=-NLL_MIN,
                            scalar2=0.0, op0=OP.add, op1=OP.max)
    kp = fin.tile([128, X], F16)
    nc.vector.tensor_scalar(out=kp[:], in0=nll[:], scalar1=NLL_MIN,
                            scalar2=None, op0=OP.is_ge)
    acc2 = fin.tile([128, 2], F32)
    nc.vector.tensor_reduce(acc2[:, 0:1], rl[:], axis=mybir.AxisListType.X,
                            op=OP.add)
    nc.vector.tensor_reduce(acc2[:, 1:2], kp[:], axis=mybir.AxisListType.X,
                            op=OP.add)

    nc.sync.dma_start(acc[:, :], acc2[:])


def build_nc(g):
    nc = bacc.Bacc(
        "TRN2",
        target_bir_lowering=False,
        debug=False,
        enable_asserts=True,
        num_devices=NCORES,
    )
    xk = nc.dram_tensor("xk", [g.NROWS, g.X], F16, kind="ExternalInput")
    x0 = nc.dram_tensor("x0", [g.NBLK, g.X], F16, kind="ExternalInput")
    maps = nc.dram_tensor("maps", [128, g.NT * 128], F16, kind="ExternalInput")
    acc = nc.dram_tensor("acc", [g.NBLK, 2], F32, kind="ExternalOutput")
    with tile.TileContext(nc) as tc, ExitStack() as ctx:
        emit(ctx, tc, g, xk.ap(), x0.ap(), maps.ap(), acc.ap())
    nc.compile()
    return nc


_NC_CACHE = {}


def _get_nc(g):
    key = (g.X, g.NBLK)
    if key not in _NC_CACHE:
        _NC_CACHE[key] = build_nc(g)
    return _NC_CACHE[key]


def make_inputs(pred_slice, target_slice, g):
    """Per-core packed inputs.  xk[b*19 + c, px] holds the fp16 logits with
    the target class swapped into slot 0; x0 is the slot-0 plane."""
    xk = np.empty((g.NBLK, C, g.X), np.float16)
    xk[:] = pred_slice.reshape(C, g.NBLK, g.X).swapaxes(0, 1)
    t = target_slice.reshape(g.NBLK, g.X).astype(np.intp)
    bi = np.arange(g.NBLK)[:, None]
    ci = np.arange(g.X)[None, :]
    xt = xk[bi, t, ci].copy()
    xk[bi, t, ci] = xk[:, 0, :]
    xk[:, 0, :] = xt
    return {"xk": xk.reshape(g.NROWS, g.X), "x0": xt}


def _shard_inputs(pred, target, g):
    """Slice the full inputs into per-core in_maps (8 cores)."""
    consts = make_consts(g)
    in_maps = []
    rows_per_core = HH // 2  # 256
    for core in range(NCORES):
        b, half = core // 2, core % 2
        h0 = half * rows_per_core
        m = make_inputs(pred[b, :, h0:h0 + rows_per_core, :],
                        target[b, h0:h0 + rows_per_core, :], g)
        m.update(consts)
        in_maps.append(m)
    return in_maps


def _reference_numpy(pred, target):
    """Full numpy fallback with reference semantics (degenerate cases only)."""
    b, c, h, w = pred.shape
    n = b * h * w
    t = target.reshape(-1).astype(np.int64)
    valid = t != 255
    t0 = np.where(valid, t, 0)
    logits = np.transpose(pred, (0, 2, 3, 1)).reshape(n, c).astype(np.float32)
    m = logits.max(axis=1, keepdims=True)
    ex = np.exp(logits - m)
    s = ex.sum(axis=1)
    pt = ex[np.arange(n), t0] / s
    mask_prob = np.where(valid, pt, 1.0).astype(np.float32)
    kth = np.sort(mask_prob)[min(n, MIN_KEPT) - 1]
    thr = max(float(kth), THRESH)
    kept = mask_prob <= thr
    fv = valid & kept
    nll = (np.log(s) + m[:, 0] - logits[np.arange(n), t0]).astype(np.float32)
    num = float(np.where(fv, nll, 0.0).sum(dtype=np.float64))
    cnt = float(fv.sum())
    return np.float32(num / max(cnt, 1.0))


def _run_device(in_maps, g, trace=False):
    nc = _get_nc(g)
    return run_bass_kernel_spmd(nc, in_maps, list(range(NCORES)), trace=trace)


def kernel(pred, target):
    pred = np.asarray(pred)
    target = np.asarray(target)
    assert pred.shape == (BATCH, C, HH, WW), pred.shape
    assert target.shape == (BATCH, HH, WW), target.shape

    if target.min() < 0 or target.max() >= C:
        # ignore_index / out-of-range labels: not producible by the input
        # spec (randint 0..18); handle via the host reference for safety.
        return _reference_numpy(pred, target)

    g = GEO_FULL
    in_maps = _shard_inputs(pred, target, g)
    res = _run_device(in_maps, g).results

    num = 0.0
    cnt = 0.0
    for core in range(NCORES):
        a = res[core]["acc"].astype(np.float64)
        num += a[:, 0].sum()
        cnt += a[:, 1].sum()
    num += NLL_MIN * cnt

    if cnt < MIN_KEPT:
        # kth-smallest prob exceeds 0.7: threshold is data-dependent.
        return _reference_numpy(pred, target)

    return np.float32(num / max(cnt, 1.0))
